# revision 12
# baseline (speedup 1.0000x reference)
"""Multi-head attention Trainium2 kernel (B=4, N=2048, D=1024, H=16).

Sharding: 8 cores = 4 batches x 2 head-groups (8 heads each), zero
collectives. Each core (all f16 compute, f32 PSUM accumulation):
  - projections: k/v + first q chunk up front; later q chunks projected
    mid-attention as PE gap filler
  - q,k kept transposed [feat, seq]; v row-layout [key, head, hd]
  - S matmuls packed two heads per pass via disjoint PE row groups into one
    [128,1024] PSUM tile
  - exp split between ACT (exact, table exp) and DVE (Schraudolph bit-trick:
    one tensor_scalar f32->int16 writing f16 bits; ~3% per-element error on
    a tunable fraction of key-tiles)
  - PV packed two heads per pass via disjoint PE col groups (M=64 each) at
    full array rate
  - softmax denominators via 4-way col-group-packed M=1 ones-matmuls
    (one PE pass covers 4 heads), reciprocal on the raw [1,512] rows,
    broadcast across partitions on the idle GPSIMD engine, normalize on DVE
  - out-projection partials staged f16; host sums the two head-group
    partials per batch and adds bias
"""
from collections import deque
from contextlib import ExitStack

import numpy as np

import concourse.mybir as mybir
import concourse.tile as tile
from concourse import bacc
from concourse.bass_utils import run_bass_kernel_spmd

F32 = mybir.dt.float32
F16 = mybir.dt.float16
I16 = mybir.dt.int16

P = 128
N = 2048         # sequence length
DI = 1024        # model dim
NH = 8           # heads per core
HD = 64          # head dim
NPAIR = 4        # head pairs per core
KT = 8           # contraction tiles for projections
CH = 512         # query chunk width
NCHUNK = 4       # chunks per sequence
MT = 16          # key tiles (m) per sequence
ET = 8           # output-feature blocks
SCALE = HD ** -0.5

LOG2E = 1.4426950408889634
A_SCH = SCALE * 1024.0 * LOG2E
B_SCH = 15.0 * 1024.0 - 55.0 + 0.5

# key-tile indices whose exp runs on DVE (Schraudolph approx); rest on ACT
DVE_EXP_MS = frozenset(m for m in range(MT) if m % 8 in (1, 4, 6))

_NC_CACHE = None


def _build():
    nc = bacc.Bacc("TRN2", target_bir_lowering=False, debug=False)

    xT = nc.dram_tensor("xT", [DI, N], F16, kind="ExternalInput").ap()
    wqkA = nc.dram_tensor("wqkA", [8, P, KT, P], F16, kind="ExternalInput").ap()
    wvA = nc.dram_tensor("wvA", [P, KT, 512], F16, kind="ExternalInput").ap()
    woT = nc.dram_tensor("woT", [512, DI], F16, kind="ExternalInput").ap()
    onesd = nc.dram_tensor("ones", [P, HD], F16, kind="ExternalInput").ap()
    seld = nc.dram_tensor("sel", [P, 2, P], F16, kind="ExternalInput").ap()
    outT = nc.dram_tensor("outT", [DI, N], F16, kind="ExternalOutput").ap()

    xT_r = xT.rearrange("(k p) n -> k p n", p=P)        # [8, 128, 2048]
    woT_r = woT.rearrange("(k p) e -> k p e", p=P)      # [4, 128, 1024]
    outT_r = outT.rearrange("(e p) n -> e p n", p=P)    # [8, 128, 2048]

    with tile.TileContext(nc) as tc, ExitStack() as persist:
        qk_pool = persist.enter_context(tc.tile_pool(name="qkp", bufs=8))
        v_pool = persist.enter_context(tc.tile_pool(name="vp", bufs=1))
        misc = persist.enter_context(tc.tile_pool(name="misc", bufs=1))
        xt_pool = persist.enter_context(tc.tile_pool(name="xt", bufs=8))
        wk_pool = persist.enter_context(tc.tile_pool(name="wk", bufs=4))
        wq_pool = persist.enter_context(tc.tile_pool(name="wq", bufs=4))
        wo_pool = persist.enter_context(tc.tile_pool(name="wo", bufs=4))

        ones_r = misc.tile([P, HD], F16)
        nc.sync.dma_start(ones_r[:], onesd[:])
        # selector weights: sel[:, i, :].T @ rd broadcasts rd row 64i across
        # partitions 0:64 and row 64i+32 across partitions 64:128
        sel_r = misc.tile([P, 2, P], F16, name="sel")
        nc.sync.dma_start(sel_r[:], seld[:])
        # warm the ACT exp table set early (one-time ~2.7us load)
        warm = misc.tile([1, 4], F16, name="warm")
        nc.scalar.activation(warm[:], ones_r[0:1, 0:4],
                             mybir.ActivationFunctionType.Exp)

        # k-projection weights first: the first matmuls need wk[0] + xt[0]
        wk = []
        for f in range(4):
            t = wk_pool.tile([P, KT, P], F16, name=f"wk{f}", tag="wk")
            nc.sync.dma_start(t[:], wqkA[4 + f])
            wk.append(t)
        xt = []
        for k in range(KT):
            t = xt_pool.tile([P, N], F16, name=f"xt{k}", tag="xt")
            nc.sync.dma_start(t[:], xT_r[k])
            xt.append(t)

        # qkT tiles: 0..3 = q head-pairs, 4..7 = k head-pairs.
        # Tile j holds heads 2j (parts 0:64) and 2j+1 (parts 64:128).
        qkT = [qk_pool.tile([P, N], F16, name=f"qkT{t}", tag="qkT")
               for t in range(8)]
        v_t = v_pool.tile([P, MT, NH, HD], F16)

        # ---------------- Phase 1: k/v projections + first q chunk --------
        with ExitStack() as ph1:
            wv_pool = ph1.enter_context(tc.tile_pool(name="wv", bufs=1))
            pp = ph1.enter_context(tc.tile_pool(name="pp", bufs=4, space="PSUM"))

            wv = wv_pool.tile([P, KT, 512], F16)
            nc.sync.dma_start(wv[:], wvA[:])
            wq = []
            for f in range(4):
                t = wq_pool.tile([P, KT, P], F16, name=f"wq{f}", tag="wq")
                nc.sync.dma_start(t[:], wqkA[f])
                wq.append(t)
            wo = [wo_pool.tile([P, DI], F16, name=f"wo{kk}", tag="wo")
                  for kk in range(NPAIR)]
            for kk in range(NPAIR):
                nc.sync.dma_start(wo[kk][:], woT_r[kk])

            # k projection: feature blocks 4..7, all chunks
            for f in range(4):
                for c in range(NCHUNK):
                    ps = pp.tile([P, CH], F32, tag="pp")
                    for k in range(KT):
                        nc.tensor.matmul(ps[:], wk[f][:, k, :],
                                         xt[k][:, c * CH:(c + 1) * CH],
                                         start=(k == 0), stop=(k == KT - 1))
                    nc.vector.tensor_copy(qkT[4 + f][:, c * CH:(c + 1) * CH],
                                          ps[:])

            # v projection: row block r
            for r in range(MT):
                ps = pp.tile([P, CH], F32, tag="pp")
                for k in range(KT):
                    nc.tensor.matmul(ps[:], xt[k][:, r * P:(r + 1) * P],
                                     wv[:, k, :],
                                     start=(k == 0), stop=(k == KT - 1))
                nc.vector.tensor_copy(v_t[:, r, :, :],
                                      ps.rearrange("p (h d) -> p h d", d=HD))

            # q projection for chunk 0 only
            for f in range(4):
                ps = pp.tile([P, CH], F32, tag="pp")
                for k in range(KT):
                    nc.tensor.matmul(ps[:], wq[f][:, k, :], xt[k][:, 0:CH],
                                     start=(k == 0), stop=(k == KT - 1))
                nc.vector.tensor_copy(qkT[f][:, 0:CH], ps[:])

        # ---------------- Phase 2: attention + out-projection -------------
        with ExitStack() as ph2:
            exp_pool = ph2.enter_context(tc.tile_pool(name="expp", bufs=32))
            osb_pool = ph2.enter_context(tc.tile_pool(name="osb", bufs=4))
            ot_pool = ph2.enter_context(tc.tile_pool(name="ot", bufs=8))
            rd_pool = ph2.enter_context(tc.tile_pool(name="rd", bufs=2))
            st_pool = ph2.enter_context(tc.tile_pool(name="stg", bufs=2))
            sps_pool = ph2.enter_context(
                tc.tile_pool(name="sps", bufs=2, space="PSUM"))
            oaug_pool = ph2.enter_context(
                tc.tile_pool(name="oaug", bufs=1, space="PSUM"))
            den_pool = ph2.enter_context(
                tc.tile_pool(name="den", bufs=1, space="PSUM"))
            aux_pool = ph2.enter_context(
                tc.tile_pool(name="aux", bufs=2, space="PSUM"))

            ot_map = {}

            def emit_s_exp(c, p):
                csl = slice(c * CH, (c + 1) * CH)
                qA = qkT[p][0:HD, csl]
                qB = qkT[p][HD:P, csl]
                kTl = qkT[4 + p]
                exps = []
                for m in range(MT):
                    msl = slice(m * P, (m + 1) * P)
                    s_ps = sps_pool.tile([P, 2 * CH], F32, tag="sps",
                                         name=f"sps_{c}_{p}_{m}")
                    nc.tensor.matmul(s_ps[:, 0:CH], kTl[0:HD, msl], qA,
                                     start=True, stop=True)
                    nc.tensor.matmul(s_ps[:, CH:2 * CH], kTl[HD:P, msl], qB,
                                     start=True, stop=True)
                    e = exp_pool.tile([P, 2 * CH], F16, tag="expp",
                                      name=f"expP_{c}_{p}_{m}")
                    if m in DVE_EXP_MS:
                        nc.vector.tensor_scalar(
                            e[:].bitcast(I16), s_ps[:], A_SCH, B_SCH,
                            mybir.AluOpType.mult, mybir.AluOpType.add)
                    else:
                        nc.scalar.activation(
                            e[:], s_ps[:], mybir.ActivationFunctionType.Exp,
                            scale=SCALE)
                    exps.append(e)
                return exps

            def emit_pv(c, p, exps, den=None, exps_prev=None):
                # odd units also emit the quad's den matmuls per-m, directly
                # after the PV pair, so the scheduler keeps each m's 6 MMs
                # adjacent and the 4 den MMs run concurrently via col groups
                oaug = oaug_pool.tile([P, CH], F32, tag="oaug",
                                      name=f"oaug_{c}_{p}")
                for m in range(MT):
                    nc.tensor.matmul(oaug[0:HD, :], v_t[:, m, 2 * p, :],
                                     exps[m][:, 0:CH],
                                     start=(m == 0), stop=(m == MT - 1))
                    nc.tensor.matmul(oaug[HD:P, :], v_t[:, m, 2 * p + 1, :],
                                     exps[m][:, CH:2 * CH],
                                     start=(m == 0), stop=(m == MT - 1))
                    if den is not None:
                        srcs = (exps_prev[m][:, 0:CH],
                                exps_prev[m][:, CH:2 * CH],
                                exps[m][:, 0:CH], exps[m][:, CH:2 * CH])
                        for j in range(4):
                            nc.tensor.matmul(den[32 * j:32 * j + 1, :],
                                             ones_r[:, 0:1], srcs[j],
                                             start=(m == 0),
                                             stop=(m == MT - 1),
                                             tile_position=(0, 32 * j))
                o_sb = osb_pool.tile([P, CH], F32, tag="osb",
                                     name=f"osb_{c}_{p}")
                nc.vector.tensor_copy(o_sb[:], oaug[:])
                return o_sb

            def emit_recip(c, q, den):
                # single full-tile reciprocal (fixed ~3.4us instruction cost
                # dominates, so one [128,512] beats four [1,512]); den rows
                # outside 32j were memset to 1.0 so recip stays finite
                rd = rd_pool.tile([P, CH], F16, tag="rd", name=f"rd_{c}_{q}")
                with nc.allow_low_precision(reason="softmax denom recip"):
                    nc.vector.reciprocal(rd[:], den[:])
                return rd

            def emit_norm(unit):
                c, q, rd, o_sbs = unit
                for i, pp_ in enumerate((2 * q, 2 * q + 1)):
                    rbc = aux_pool.tile([P, CH], F32, tag="aux",
                                        name=f"rbc_{c}_{q}_{i}")
                    nc.tensor.matmul(rbc[:], sel_r[:, i, :], rd[:],
                                     start=True, stop=True)
                    ot_p = ot_pool.tile([P, CH], F16, tag="ot",
                                        name=f"ot_{c}_{pp_}")
                    nc.vector.tensor_tensor(ot_p[:], o_sbs[i][:], rbc[:],
                                            mybir.AluOpType.mult)
                    ot_map[(c, pp_)] = ot_p

            def emit_qproj(c, f):
                csl = slice(c * CH, (c + 1) * CH)
                ps = aux_pool.tile([P, CH], F32, tag="aux",
                                   name=f"qp_{c}_{f}")
                for k in range(KT):
                    nc.tensor.matmul(ps[:], wq[f][:, k, :], xt[k][:, csl],
                                     start=(k == 0), stop=(k == KT - 1))
                nc.vector.tensor_copy(qkT[f][:, csl], ps[:])

            def emit_outproj(c):
                csl = slice(c * CH, (c + 1) * CH)
                for e in range(ET):
                    pso = aux_pool.tile([P, CH], F32, tag="aux",
                                        name=f"pso_{c}_{e}")
                    for p in range(NPAIR):
                        nc.tensor.matmul(pso[:],
                                         wo[p][:, e * P:(e + 1) * P],
                                         ot_map[(c, p)][:],
                                         start=(p == 0), stop=(p == NPAIR - 1))
                    st = st_pool.tile([P, CH], F16, tag="stg",
                                      name=f"st_{c}_{e}")
                    nc.vector.tensor_copy(st[:], pso[:])
                    nc.sync.dma_start(outT_r[e][:, csl], st[:])

            # software pipeline: quad q's norm is emitted one unit after its
            # den completes; chunk c's out-projection after chunk c+1's first
            # unit, so the PE queue never blocks on the DVE/GPSIMD chain
            pend = deque()
            exps_even = None
            for c in range(NCHUNK):
                for p in range(NPAIR):
                    exps = emit_s_exp(c, p)
                    if p % 2 == 0:
                        o_sb_even = emit_pv(c, p, exps)
                        exps_even = exps
                    else:
                        q = p // 2
                        den = den_pool.tile([P, CH], F32, tag="den",
                                            name=f"den_{c}_{q}")
                        nc.vector.memset(den[:], 1.0)
                        o_sb = emit_pv(c, p, exps, den=den,
                                       exps_prev=exps_even)
                        rd = emit_recip(c, q, den)
                        pend.append((c, q, rd, (o_sb_even, o_sb)))
                        exps_even = None
                    if c + 1 < NCHUNK:
                        emit_qproj(c + 1, p)
                    if len(pend) > 1:
                        emit_norm(pend.popleft())
                    if p == 2 and c > 0:
                        while pend and pend[0][0] < c:
                            emit_norm(pend.popleft())
                        emit_outproj(c - 1)
            while pend:
                emit_norm(pend.popleft())
            emit_outproj(NCHUNK - 1)

    nc.compile()
    return nc


def _get_nc():
    global _NC_CACHE
    if _NC_CACHE is None:
        _NC_CACHE = _build()
    return _NC_CACHE


def _make_in_maps(x, w_qkv, w_out):
    ones = np.ones((P, HD), dtype=np.float16)
    sel = np.zeros((P, 2, P), dtype=np.float16)
    for i in range(2):
        sel[64 * i, i, 0:64] = 1.0
        sel[64 * i + 32, i, 64:128] = 1.0
    per_g = []
    for g in range(2):
        qk_g = np.concatenate([w_qkv[g * 512:(g + 1) * 512],
                               w_qkv[DI + g * 512:DI + (g + 1) * 512]], axis=0)
        wqkT = np.ascontiguousarray(qk_g.T)               # [1024 d, 1024 f]
        wqkA = np.ascontiguousarray(
            wqkT.reshape(KT, P, 8, P).transpose(2, 1, 0, 3).astype(np.float16))
        v_g = w_qkv[2 * DI + g * 512:2 * DI + (g + 1) * 512]
        wvT = np.ascontiguousarray(v_g.T)                 # [1024 d, 512 f]
        wvA = np.ascontiguousarray(
            wvT.reshape(KT, P, 512).transpose(1, 0, 2).astype(np.float16))
        woTg = np.ascontiguousarray(
            w_out[:, g * 512:(g + 1) * 512].T.astype(np.float16))
        per_g.append((wqkA, wvA, woTg))

    in_maps = []
    for c in range(8):
        b, g = c // 2, c % 2
        wqkA, wvA, woTg = per_g[g]
        in_maps.append({
            "xT": np.ascontiguousarray(x[b].T.astype(np.float16)),
            "wqkA": wqkA,
            "wvA": wvA,
            "woT": woTg,
            "ones": ones,
            "sel": sel,
        })
    return in_maps


def kernel(x, w_qkv, w_out, b_out):
    x = np.asarray(x, dtype=np.float32)
    w_qkv = np.asarray(w_qkv, dtype=np.float32)
    w_out = np.asarray(w_out, dtype=np.float32)
    b_out = np.asarray(b_out, dtype=np.float32)
    B = x.shape[0]

    in_maps = _make_in_maps(x, w_qkv, w_out)
    nc = _get_nc()
    res = run_bass_kernel_spmd(nc, in_maps, core_ids=list(range(8)))
    parts = [r["outT"] for r in res.results]
    out = np.empty((B, N, DI), dtype=np.float32)
    for b in range(B):
        out[b] = (parts[2 * b].astype(np.float32)
                  + parts[2 * b + 1].astype(np.float32)).T + b_out
    return out


# revision 13
# speedup vs baseline: 1.2198x; 1.2198x over previous
"""Multi-head attention Trainium2 kernel (B=4, N=2048, D=1024, H=16).

Sharding: 8 cores = 4 batches x 2 head-groups (8 heads each), zero
collectives. Each core (all f16 compute, f32 PSUM accumulation):
  - projections: k/v + first q chunk up front; later q chunks projected
    mid-attention as PE gap filler
  - q,k kept transposed [feat, seq]; v row-layout [key, head, hd]
  - S matmuls packed two heads per pass via disjoint PE row groups into one
    [128,1024] PSUM tile
  - exp split between ACT (exact, table exp) and DVE (Schraudolph bit-trick:
    one tensor_scalar f32->int16 writing f16 bits; ~3% per-element error on
    a tunable fraction of key-tiles)
  - PV packed two heads per pass via disjoint PE col groups (M=64 each) at
    full array rate
  - softmax denominators via 4-way col-group-packed M=1 ones-matmuls
    (one PE pass covers 4 heads), reciprocal on the raw [1,512] rows,
    broadcast across partitions on the idle GPSIMD engine, normalize on DVE
  - out-projection partials staged f16; host sums the two head-group
    partials per batch and adds bias
"""
from collections import deque
from contextlib import ExitStack

import numpy as np

import concourse.mybir as mybir
import concourse.tile as tile
from concourse import bacc
from concourse.bass_utils import run_bass_kernel_spmd

F32 = mybir.dt.float32
F16 = mybir.dt.float16
I16 = mybir.dt.int16

P = 128
N = 2048         # sequence length
DI = 1024        # model dim
NH = 8           # heads per core
HD = 64          # head dim
NPAIR = 4        # head pairs per core
KT = 8           # contraction tiles for projections
CH = 512         # query chunk width
NCHUNK = 4       # chunks per sequence
MT = 16          # key tiles (m) per sequence
ET = 8           # output-feature blocks
SCALE = HD ** -0.5

LOG2E = 1.4426950408889634
A_SCH = SCALE * 1024.0 * LOG2E
B_SCH = 15.0 * 1024.0 - 55.0 + 0.5

# key-tile indices whose exp runs on DVE (Schraudolph approx); rest on ACT
DVE_EXP_MS = frozenset(m for m in range(MT) if m % 8 in (1, 4, 6))

_NC_CACHE = None


def _build():
    nc = bacc.Bacc("TRN2", target_bir_lowering=False, debug=False)

    xT = nc.dram_tensor("xT", [DI, N], F16, kind="ExternalInput").ap()
    wqkA = nc.dram_tensor("wqkA", [8, P, KT, P], F16, kind="ExternalInput").ap()
    wvA = nc.dram_tensor("wvA", [P, KT, 512], F16, kind="ExternalInput").ap()
    woT = nc.dram_tensor("woT", [512, DI], F16, kind="ExternalInput").ap()
    onesd = nc.dram_tensor("ones", [P, HD], F16, kind="ExternalInput").ap()
    seld = nc.dram_tensor("sel", [P, 2, P], F16, kind="ExternalInput").ap()
    outT = nc.dram_tensor("outT", [DI, N], F16, kind="ExternalOutput").ap()

    xT_r = xT.rearrange("(k p) n -> k p n", p=P)        # [8, 128, 2048]
    woT_r = woT.rearrange("(k p) e -> k p e", p=P)      # [4, 128, 1024]
    outT_r = outT.rearrange("(e p) n -> e p n", p=P)    # [8, 128, 2048]

    with tile.TileContext(nc) as tc, ExitStack() as persist:
        qk_pool = persist.enter_context(tc.tile_pool(name="qkp", bufs=8))
        v_pool = persist.enter_context(tc.tile_pool(name="vp", bufs=1))
        misc = persist.enter_context(tc.tile_pool(name="misc", bufs=1))
        xt_pool = persist.enter_context(tc.tile_pool(name="xt", bufs=8))
        wk_pool = persist.enter_context(tc.tile_pool(name="wk", bufs=4))
        wq_pool = persist.enter_context(tc.tile_pool(name="wq", bufs=4))
        wo_pool = persist.enter_context(tc.tile_pool(name="wo", bufs=4))

        ones_r = misc.tile([P, HD], F16)
        nc.sync.dma_start(ones_r[:], onesd[:])
        # selector weights: sel[:, i, :].T @ rd broadcasts rd row 64i across
        # partitions 0:64 and row 64i+32 across partitions 64:128
        sel_r = misc.tile([P, 2, P], F16, name="sel")
        nc.sync.dma_start(sel_r[:], seld[:])
        # warm the ACT exp table set early (one-time ~2.7us load)
        warm = misc.tile([1, 4], F16, name="warm")
        nc.scalar.activation(warm[:], ones_r[0:1, 0:4],
                             mybir.ActivationFunctionType.Exp)

        # k-projection weights first: the first matmuls need wk[0] + xt[0]
        wk = []
        for f in range(4):
            t = wk_pool.tile([P, KT, P], F16, name=f"wk{f}", tag="wk")
            nc.sync.dma_start(t[:], wqkA[4 + f])
            wk.append(t)
        xt = []
        for k in range(KT):
            t = xt_pool.tile([P, N], F16, name=f"xt{k}", tag="xt")
            nc.sync.dma_start(t[:], xT_r[k])
            xt.append(t)

        # qkT tiles: 0..3 = q head-pairs, 4..7 = k head-pairs.
        # Tile j holds heads 2j (parts 0:64) and 2j+1 (parts 64:128).
        qkT = [qk_pool.tile([P, N], F16, name=f"qkT{t}", tag="qkT")
               for t in range(8)]
        v_t = v_pool.tile([P, MT, NH, HD + 1], F16)
        nc.vector.tensor_copy(v_t[:, :, :, HD:HD + 1],
                              ones_r[:, 0:1].to_broadcast((P, MT, NH, 1)))

        # ---------------- Phase 1: k/v projections + first q chunk --------
        with ExitStack() as ph1:
            wv_pool = ph1.enter_context(tc.tile_pool(name="wv", bufs=1))
            pp = ph1.enter_context(tc.tile_pool(name="pp", bufs=4, space="PSUM"))

            wv = wv_pool.tile([P, KT, 512], F16)
            nc.sync.dma_start(wv[:], wvA[:])
            wq = []
            for f in range(4):
                t = wq_pool.tile([P, KT, P], F16, name=f"wq{f}", tag="wq")
                nc.sync.dma_start(t[:], wqkA[f])
                wq.append(t)
            wo = [wo_pool.tile([P, DI], F16, name=f"wo{kk}", tag="wo")
                  for kk in range(NPAIR)]
            for kk in range(NPAIR):
                nc.sync.dma_start(wo[kk][:], woT_r[kk])

            # k projection: feature blocks 4..7, all chunks
            for f in range(4):
                for c in range(NCHUNK):
                    ps = pp.tile([P, CH], F32, tag="pp")
                    for k in range(KT):
                        nc.tensor.matmul(ps[:], wk[f][:, k, :],
                                         xt[k][:, c * CH:(c + 1) * CH],
                                         start=(k == 0), stop=(k == KT - 1))
                    nc.vector.tensor_copy(qkT[4 + f][:, c * CH:(c + 1) * CH],
                                          ps[:])

            # v projection: row block r
            for r in range(MT):
                ps = pp.tile([P, CH], F32, tag="pp")
                for k in range(KT):
                    nc.tensor.matmul(ps[:], xt[k][:, r * P:(r + 1) * P],
                                     wv[:, k, :],
                                     start=(k == 0), stop=(k == KT - 1))
                nc.vector.tensor_copy(v_t[:, r, :, 0:HD],
                                      ps.rearrange("p (h d) -> p h d", d=HD))

            # q projection for chunk 0 only
            for f in range(4):
                ps = pp.tile([P, CH], F32, tag="pp")
                for k in range(KT):
                    nc.tensor.matmul(ps[:], wq[f][:, k, :], xt[k][:, 0:CH],
                                     start=(k == 0), stop=(k == KT - 1))
                nc.vector.tensor_copy(qkT[f][:, 0:CH], ps[:])

        # ---------------- Phase 2: attention + out-projection -------------
        with ExitStack() as ph2:
            exp_pool = ph2.enter_context(tc.tile_pool(name="expp", bufs=32))
            osb_pool = ph2.enter_context(tc.tile_pool(name="osb", bufs=4))
            ot_pool = ph2.enter_context(tc.tile_pool(name="ot", bufs=8))
            rd_pool = ph2.enter_context(tc.tile_pool(name="rd", bufs=2))
            st_pool = ph2.enter_context(tc.tile_pool(name="stg", bufs=2))
            sps_pool = ph2.enter_context(
                tc.tile_pool(name="sps", bufs=2, space="PSUM"))
            oaug_pool = ph2.enter_context(
                tc.tile_pool(name="oaug", bufs=1, space="PSUM"))
            aux_pool = ph2.enter_context(
                tc.tile_pool(name="aux", bufs=2, space="PSUM"))

            ot_map = {}

            def emit_s_exp(c, p):
                csl = slice(c * CH, (c + 1) * CH)
                qA = qkT[p][0:HD, csl]
                qB = qkT[p][HD:P, csl]
                kTl = qkT[4 + p]
                exps = []
                for m in range(MT):
                    msl = slice(m * P, (m + 1) * P)
                    s_ps = sps_pool.tile([P, 2 * CH], F32, tag="sps",
                                         name=f"sps_{c}_{p}_{m}")
                    nc.tensor.matmul(s_ps[:, 0:CH], kTl[0:HD, msl], qA,
                                     start=True, stop=True)
                    nc.tensor.matmul(s_ps[:, CH:2 * CH], kTl[HD:P, msl], qB,
                                     start=True, stop=True)
                    e = exp_pool.tile([P, 2 * CH], F16, tag="expp",
                                      name=f"expP_{c}_{p}_{m}")
                    if m in DVE_EXP_MS:
                        nc.vector.tensor_scalar(
                            e[:].bitcast(I16), s_ps[:], A_SCH, B_SCH,
                            mybir.AluOpType.mult, mybir.AluOpType.add)
                    else:
                        nc.scalar.activation(
                            e[:], s_ps[:], mybir.ActivationFunctionType.Exp,
                            scale=SCALE)
                    exps.append(e)
                return exps

            def emit_pv(c, p, exps):
                # M=65 aug: the ones column of v makes output row 64 the
                # softmax denominator for free; two heads sequential
                o_sbs = []
                for h in range(2):
                    oaug = oaug_pool.tile([HD + 1, CH], F32, tag="oaug",
                                          name=f"oaug_{c}_{p}_{h}")
                    for m in range(MT):
                        nc.tensor.matmul(oaug[:], v_t[:, m, 2 * p + h, :],
                                         exps[m][:, h * CH:(h + 1) * CH],
                                         start=(m == 0), stop=(m == MT - 1))
                    o_sb = osb_pool.tile([HD + 1, CH], F32, tag="osb",
                                         name=f"osb_{c}_{p}_{h}")
                    nc.vector.tensor_copy(o_sb[:], oaug[:])
                    o_sbs.append(o_sb)
                return o_sbs

            def emit_recip(c, q, o_sb4):
                # gather the 4 heads' raw denominators (row 64 of each evac)
                # into one tile, then a single reciprocal: the instruction's
                # ~3.4us fixed cost dominates, so one [128,512] beats four
                rd = rd_pool.tile([P, CH], F16, tag="rd", name=f"rdr_{c}_{q}")
                nc.vector.memset(rd[:], 1.0)
                for j, osb in enumerate(o_sb4):
                    nc.vector.tensor_copy(rd[32 * j:32 * j + 1, :],
                                          osb[HD:HD + 1, :])
                rd2 = rd_pool.tile([P, CH], F16, tag="rd2", name=f"rd_{c}_{q}")
                with nc.allow_low_precision(reason="softmax denom recip"):
                    nc.vector.reciprocal(rd2[:], rd[:])
                return rd2

            def emit_norm(unit):
                c, q, rd, o_sb4 = unit
                for i, pp_ in enumerate((2 * q, 2 * q + 1)):
                    rbc = aux_pool.tile([P, CH], F32, tag="aux",
                                        name=f"rbc_{c}_{q}_{i}")
                    nc.tensor.matmul(rbc[:], sel_r[:, i, :], rd[:],
                                     start=True, stop=True)
                    ot_p = ot_pool.tile([P, CH], F16, tag="ot",
                                        name=f"ot_{c}_{pp_}")
                    nc.vector.tensor_tensor(ot_p[0:HD, :],
                                            o_sb4[2 * i][0:HD, :],
                                            rbc[0:HD, :],
                                            mybir.AluOpType.mult)
                    nc.vector.tensor_tensor(ot_p[HD:P, :],
                                            o_sb4[2 * i + 1][0:HD, :],
                                            rbc[HD:P, :],
                                            mybir.AluOpType.mult)
                    ot_map[(c, pp_)] = ot_p

            def emit_qproj(c, f):
                csl = slice(c * CH, (c + 1) * CH)
                ps = aux_pool.tile([P, CH], F32, tag="aux",
                                   name=f"qp_{c}_{f}")
                for k in range(KT):
                    nc.tensor.matmul(ps[:], wq[f][:, k, :], xt[k][:, csl],
                                     start=(k == 0), stop=(k == KT - 1))
                nc.vector.tensor_copy(qkT[f][:, csl], ps[:])

            def emit_outproj(c):
                csl = slice(c * CH, (c + 1) * CH)
                for e in range(ET):
                    pso = aux_pool.tile([P, CH], F32, tag="aux",
                                        name=f"pso_{c}_{e}")
                    for p in range(NPAIR):
                        nc.tensor.matmul(pso[:],
                                         wo[p][:, e * P:(e + 1) * P],
                                         ot_map[(c, p)][:],
                                         start=(p == 0), stop=(p == NPAIR - 1))
                    st = st_pool.tile([P, CH], F16, tag="stg",
                                      name=f"st_{c}_{e}")
                    nc.scalar.copy(st[:], pso[:])
                    nc.sync.dma_start(outT_r[e][:, csl], st[:])

            # software pipeline: quad q's norm is emitted one unit after its
            # den completes; chunk c's out-projection after chunk c+1's first
            # unit, so the PE queue never blocks on the DVE/GPSIMD chain
            pend = deque()
            exps_even = None
            for c in range(NCHUNK):
                for p in range(NPAIR):
                    exps = emit_s_exp(c, p)
                    o_sbs = emit_pv(c, p, exps)
                    if p % 2 == 0:
                        o_sbs_even = o_sbs
                    else:
                        q = p // 2
                        o_sb4 = (o_sbs_even[0], o_sbs_even[1],
                                 o_sbs[0], o_sbs[1])
                        rd = emit_recip(c, q, o_sb4)
                        pend.append((c, q, rd, o_sb4))
                    if c + 1 < NCHUNK:
                        emit_qproj(c + 1, p)
                    if len(pend) > 1:
                        emit_norm(pend.popleft())
                    if p == 2 and c > 0:
                        while pend and pend[0][0] < c:
                            emit_norm(pend.popleft())
                        emit_outproj(c - 1)
            while pend:
                emit_norm(pend.popleft())
            emit_outproj(NCHUNK - 1)

    nc.compile()
    return nc


def _get_nc():
    global _NC_CACHE
    if _NC_CACHE is None:
        _NC_CACHE = _build()
    return _NC_CACHE


def _make_in_maps(x, w_qkv, w_out):
    ones = np.ones((P, HD), dtype=np.float16)
    sel = np.zeros((P, 2, P), dtype=np.float16)
    for i in range(2):
        sel[64 * i, i, 0:64] = 1.0
        sel[64 * i + 32, i, 64:128] = 1.0
    per_g = []
    for g in range(2):
        qk_g = np.concatenate([w_qkv[g * 512:(g + 1) * 512],
                               w_qkv[DI + g * 512:DI + (g + 1) * 512]], axis=0)
        wqkT = np.ascontiguousarray(qk_g.T)               # [1024 d, 1024 f]
        wqkA = np.ascontiguousarray(
            wqkT.reshape(KT, P, 8, P).transpose(2, 1, 0, 3).astype(np.float16))
        v_g = w_qkv[2 * DI + g * 512:2 * DI + (g + 1) * 512]
        wvT = np.ascontiguousarray(v_g.T)                 # [1024 d, 512 f]
        wvA = np.ascontiguousarray(
            wvT.reshape(KT, P, 512).transpose(1, 0, 2).astype(np.float16))
        woTg = np.ascontiguousarray(
            w_out[:, g * 512:(g + 1) * 512].T.astype(np.float16))
        per_g.append((wqkA, wvA, woTg))

    in_maps = []
    for c in range(8):
        b, g = c // 2, c % 2
        wqkA, wvA, woTg = per_g[g]
        in_maps.append({
            "xT": np.ascontiguousarray(x[b].T.astype(np.float16)),
            "wqkA": wqkA,
            "wvA": wvA,
            "woT": woTg,
            "ones": ones,
            "sel": sel,
        })
    return in_maps


def kernel(x, w_qkv, w_out, b_out):
    x = np.asarray(x, dtype=np.float32)
    w_qkv = np.asarray(w_qkv, dtype=np.float32)
    w_out = np.asarray(w_out, dtype=np.float32)
    b_out = np.asarray(b_out, dtype=np.float32)
    B = x.shape[0]

    in_maps = _make_in_maps(x, w_qkv, w_out)
    nc = _get_nc()
    res = run_bass_kernel_spmd(nc, in_maps, core_ids=list(range(8)))
    parts = [r["outT"] for r in res.results]
    out = np.empty((B, N, DI), dtype=np.float32)
    for b in range(B):
        out[b] = (parts[2 * b].astype(np.float32)
                  + parts[2 * b + 1].astype(np.float32)).T + b_out
    return out


# revision 14
# speedup vs baseline: 1.2308x; 1.0090x over previous
"""Multi-head attention Trainium2 kernel (B=4, N=2048, D=1024, H=16).

Sharding: 8 cores = 4 batches x 2 head-groups (8 heads each), zero
collectives. Each core (all f16 compute, f32 PSUM accumulation):
  - projections: k/v + first q chunk up front; later q chunks projected
    mid-attention as PE gap filler
  - q,k kept transposed [feat, seq]; v row-layout [key, head, hd]
  - S matmuls packed two heads per pass via disjoint PE row groups into one
    [128,1024] PSUM tile
  - exp split between ACT (exact, table exp) and DVE (Schraudolph bit-trick:
    one tensor_scalar f32->int16 writing f16 bits; ~3% per-element error on
    a tunable fraction of key-tiles)
  - PV packed two heads per pass via disjoint PE col groups (M=64 each) at
    full array rate
  - softmax denominators via 4-way col-group-packed M=1 ones-matmuls
    (one PE pass covers 4 heads), reciprocal on the raw [1,512] rows,
    broadcast across partitions on the idle GPSIMD engine, normalize on DVE
  - out-projection partials staged f16; host sums the two head-group
    partials per batch and adds bias
"""
from collections import deque
from contextlib import ExitStack

import numpy as np

import concourse.mybir as mybir
import concourse.tile as tile
from concourse import bacc
from concourse.bass_utils import run_bass_kernel_spmd

F32 = mybir.dt.float32
F16 = mybir.dt.float16
I16 = mybir.dt.int16

P = 128
N = 2048         # sequence length
DI = 1024        # model dim
NH = 8           # heads per core
HD = 64          # head dim
NPAIR = 4        # head pairs per core
KT = 8           # contraction tiles for projections
CH = 512         # query chunk width
NCHUNK = 4       # chunks per sequence
MT = 16          # key tiles (m) per sequence
ET = 8           # output-feature blocks
SCALE = HD ** -0.5

LOG2E = 1.4426950408889634
A_SCH = SCALE * 1024.0 * LOG2E
B_SCH = 15.0 * 1024.0 - 55.0 + 0.5

# key-tile indices whose exp runs on DVE (Schraudolph approx); rest on ACT
DVE_EXP_MS = frozenset(m for m in range(MT) if m % 8 in (1, 4, 6))

_NC_CACHE = None


def _build():
    nc = bacc.Bacc("TRN2", target_bir_lowering=False, debug=False)

    xT = nc.dram_tensor("xT", [DI, N], F16, kind="ExternalInput").ap()
    wqkA = nc.dram_tensor("wqkA", [8, P, KT, P], F16, kind="ExternalInput").ap()
    wvA = nc.dram_tensor("wvA", [P, KT, 512], F16, kind="ExternalInput").ap()
    woT = nc.dram_tensor("woT", [512, DI], F16, kind="ExternalInput").ap()
    onesd = nc.dram_tensor("ones", [P, HD], F16, kind="ExternalInput").ap()
    seld = nc.dram_tensor("sel", [P, 2, P], F16, kind="ExternalInput").ap()
    outT = nc.dram_tensor("outT", [DI, N], F16, kind="ExternalOutput").ap()

    xT_r = xT.rearrange("(k p) n -> k p n", p=P)        # [8, 128, 2048]
    woT_r = woT.rearrange("(k p) e -> k p e", p=P)      # [4, 128, 1024]
    outT_r = outT.rearrange("(e p) n -> e p n", p=P)    # [8, 128, 2048]

    with tile.TileContext(nc) as tc, ExitStack() as persist:
        qk_pool = persist.enter_context(tc.tile_pool(name="qkp", bufs=8))
        v_pool = persist.enter_context(tc.tile_pool(name="vp", bufs=1))
        misc = persist.enter_context(tc.tile_pool(name="misc", bufs=1))
        xt_pool = persist.enter_context(tc.tile_pool(name="xt", bufs=8))
        wk_pool = persist.enter_context(tc.tile_pool(name="wk", bufs=4))
        wq_pool = persist.enter_context(tc.tile_pool(name="wq", bufs=4))
        wo_pool = persist.enter_context(tc.tile_pool(name="wo", bufs=4))

        ones_r = misc.tile([P, HD], F16)
        nc.sync.dma_start(ones_r[:], onesd[:])
        # selector weights: sel[:, i, :].T @ rd broadcasts rd row 64i across
        # partitions 0:64 and row 64i+32 across partitions 64:128
        sel_r = misc.tile([P, 2, P], F16, name="sel")
        nc.sync.dma_start(sel_r[:], seld[:])
        # warm the ACT exp table set early (one-time ~2.7us load)
        warm = misc.tile([1, 4], F16, name="warm")
        nc.scalar.activation(warm[:], ones_r[0:1, 0:4],
                             mybir.ActivationFunctionType.Exp)

        # DMA order: wk0 then x tiles (first matmul group needs wk0 + all
        # xt), with the remaining small weight tiles interleaved
        wk = [wk_pool.tile([P, KT, P], F16, name=f"wk{f}", tag="wk")
              for f in range(4)]
        xt = [xt_pool.tile([P, N], F16, name=f"xt{k}", tag="xt")
              for k in range(KT)]
        nc.sync.dma_start(wk[0][:], wqkA[4])
        for k in range(KT):
            nc.sync.dma_start(xt[k][:], xT_r[k])
            if 1 <= k <= 3:
                nc.sync.dma_start(wk[k][:], wqkA[4 + k])

        # qkT tiles: 0..3 = q head-pairs, 4..7 = k head-pairs.
        # Tile j holds heads 2j (parts 0:64) and 2j+1 (parts 64:128).
        qkT = [qk_pool.tile([P, N], F16, name=f"qkT{t}", tag="qkT")
               for t in range(8)]
        v_t = v_pool.tile([P, MT, NH, HD + 1], F16)
        nc.vector.tensor_copy(v_t[:, :, :, HD:HD + 1],
                              ones_r[:, 0:1].to_broadcast((P, MT, NH, 1)))

        # ---------------- Phase 1: k/v projections + first q chunk --------
        with ExitStack() as ph1:
            wv_pool = ph1.enter_context(tc.tile_pool(name="wv", bufs=1))
            pp = ph1.enter_context(tc.tile_pool(name="pp", bufs=4, space="PSUM"))

            wv = wv_pool.tile([P, KT, 512], F16)
            nc.sync.dma_start(wv[:], wvA[:])
            wq = []
            for f in range(4):
                t = wq_pool.tile([P, KT, P], F16, name=f"wq{f}", tag="wq")
                nc.sync.dma_start(t[:], wqkA[f])
                wq.append(t)
            wo = [wo_pool.tile([P, DI], F16, name=f"wo{kk}", tag="wo")
                  for kk in range(NPAIR)]
            for kk in range(NPAIR):
                nc.sync.dma_start(wo[kk][:], woT_r[kk])

            # k projection: feature blocks 4..7, all chunks
            for f in range(4):
                for c in range(NCHUNK):
                    ps = pp.tile([P, CH], F32, tag="pp")
                    for k in range(KT):
                        nc.tensor.matmul(ps[:], wk[f][:, k, :],
                                         xt[k][:, c * CH:(c + 1) * CH],
                                         start=(k == 0), stop=(k == KT - 1))
                    nc.vector.tensor_copy(qkT[4 + f][:, c * CH:(c + 1) * CH],
                                          ps[:])

            # v projection: row block r
            for r in range(MT):
                ps = pp.tile([P, CH], F32, tag="pp")
                for k in range(KT):
                    nc.tensor.matmul(ps[:], xt[k][:, r * P:(r + 1) * P],
                                     wv[:, k, :],
                                     start=(k == 0), stop=(k == KT - 1))
                nc.vector.tensor_copy(v_t[:, r, :, 0:HD],
                                      ps.rearrange("p (h d) -> p h d", d=HD))

            # q projection for chunk 0 only
            for f in range(4):
                ps = pp.tile([P, CH], F32, tag="pp")
                for k in range(KT):
                    nc.tensor.matmul(ps[:], wq[f][:, k, :], xt[k][:, 0:CH],
                                     start=(k == 0), stop=(k == KT - 1))
                nc.vector.tensor_copy(qkT[f][:, 0:CH], ps[:])

        # ---------------- Phase 2: attention + out-projection -------------
        with ExitStack() as ph2:
            exp_pool = ph2.enter_context(tc.tile_pool(name="expp", bufs=32))
            osb_pool = ph2.enter_context(tc.tile_pool(name="osb", bufs=4))
            ot_pool = ph2.enter_context(tc.tile_pool(name="ot", bufs=8))
            rd_pool = ph2.enter_context(tc.tile_pool(name="rd", bufs=2))
            st_pool = ph2.enter_context(tc.tile_pool(name="stg", bufs=2))
            sps_pool = ph2.enter_context(
                tc.tile_pool(name="sps", bufs=2, space="PSUM"))
            oaug_pool = ph2.enter_context(
                tc.tile_pool(name="oaug", bufs=1, space="PSUM"))
            aux_pool = ph2.enter_context(
                tc.tile_pool(name="aux", bufs=2, space="PSUM"))

            ot_map = {}

            def emit_s_exp(c, p):
                csl = slice(c * CH, (c + 1) * CH)
                qA = qkT[p][0:HD, csl]
                qB = qkT[p][HD:P, csl]
                kTl = qkT[4 + p]
                exps = []
                for m in range(MT):
                    msl = slice(m * P, (m + 1) * P)
                    s_ps = sps_pool.tile([P, 2 * CH], F32, tag="sps",
                                         name=f"sps_{c}_{p}_{m}")
                    nc.tensor.matmul(s_ps[:, 0:CH], kTl[0:HD, msl], qA,
                                     start=True, stop=True)
                    nc.tensor.matmul(s_ps[:, CH:2 * CH], kTl[HD:P, msl], qB,
                                     start=True, stop=True)
                    e = exp_pool.tile([P, 2 * CH], F16, tag="expp",
                                      name=f"expP_{c}_{p}_{m}")
                    if m in DVE_EXP_MS:
                        nc.vector.tensor_scalar(
                            e[:].bitcast(I16), s_ps[:], A_SCH, B_SCH,
                            mybir.AluOpType.mult, mybir.AluOpType.add)
                    else:
                        nc.scalar.activation(
                            e[:], s_ps[:], mybir.ActivationFunctionType.Exp,
                            scale=SCALE)
                    exps.append(e)
                return exps

            def emit_pv(c, p, exps):
                # M=65 aug: the ones column of v makes output row 64 the
                # softmax denominator for free; two heads sequential
                o_sbs = []
                for h in range(2):
                    oaug = oaug_pool.tile([HD + 1, CH], F32, tag="oaug",
                                          name=f"oaug_{c}_{p}_{h}")
                    for m in range(MT):
                        nc.tensor.matmul(oaug[:], v_t[:, m, 2 * p + h, :],
                                         exps[m][:, h * CH:(h + 1) * CH],
                                         start=(m == 0), stop=(m == MT - 1))
                    o_sb = osb_pool.tile([HD + 1, CH], F32, tag="osb",
                                         name=f"osb_{c}_{p}_{h}")
                    nc.scalar.copy(o_sb[:], oaug[:])
                    o_sbs.append(o_sb)
                return o_sbs

            def emit_recip(c, q, o_sb4):
                # gather the 4 heads' raw denominators (row 64 of each evac)
                # into one tile, then a single reciprocal: the instruction's
                # ~3.4us fixed cost dominates, so one [128,512] beats four
                rd = rd_pool.tile([P, CH], F16, tag="rd", name=f"rdr_{c}_{q}")
                nc.vector.memset(rd[:], 1.0)
                for j, osb in enumerate(o_sb4):
                    nc.vector.tensor_copy(rd[32 * j:32 * j + 1, :],
                                          osb[HD:HD + 1, :])
                rd2 = rd_pool.tile([P, CH], F16, tag="rd2", name=f"rd_{c}_{q}")
                with nc.allow_low_precision(reason="softmax denom recip"):
                    nc.vector.reciprocal(rd2[:], rd[:])
                return rd2

            def emit_norm(unit):
                c, q, rd, o_sb4 = unit
                for i, pp_ in enumerate((2 * q, 2 * q + 1)):
                    rbc = aux_pool.tile([P, CH], F32, tag="aux",
                                        name=f"rbc_{c}_{q}_{i}")
                    nc.tensor.matmul(rbc[:], sel_r[:, i, :], rd[:],
                                     start=True, stop=True)
                    ot_p = ot_pool.tile([P, CH], F16, tag="ot",
                                        name=f"ot_{c}_{pp_}")
                    nc.vector.tensor_tensor(ot_p[0:HD, :],
                                            o_sb4[2 * i][0:HD, :],
                                            rbc[0:HD, :],
                                            mybir.AluOpType.mult)
                    nc.vector.tensor_tensor(ot_p[HD:P, :],
                                            o_sb4[2 * i + 1][0:HD, :],
                                            rbc[HD:P, :],
                                            mybir.AluOpType.mult)
                    ot_map[(c, pp_)] = ot_p

            def emit_qproj(c, f):
                csl = slice(c * CH, (c + 1) * CH)
                ps = aux_pool.tile([P, CH], F32, tag="aux",
                                   name=f"qp_{c}_{f}")
                for k in range(KT):
                    nc.tensor.matmul(ps[:], wq[f][:, k, :], xt[k][:, csl],
                                     start=(k == 0), stop=(k == KT - 1))
                nc.vector.tensor_copy(qkT[f][:, csl], ps[:])

            def emit_outproj(c):
                csl = slice(c * CH, (c + 1) * CH)
                for e in range(ET):
                    pso = aux_pool.tile([P, CH], F32, tag="aux",
                                        name=f"pso_{c}_{e}")
                    for p in range(NPAIR):
                        nc.tensor.matmul(pso[:],
                                         wo[p][:, e * P:(e + 1) * P],
                                         ot_map[(c, p)][:],
                                         start=(p == 0), stop=(p == NPAIR - 1))
                    st = st_pool.tile([P, CH], F16, tag="stg",
                                      name=f"st_{c}_{e}")
                    nc.scalar.copy(st[:], pso[:])
                    nc.sync.dma_start(outT_r[e][:, csl], st[:])

            # software pipeline: quad q's norm is emitted one unit after its
            # den completes; chunk c's out-projection after chunk c+1's first
            # unit, so the PE queue never blocks on the DVE/GPSIMD chain
            pend = deque()
            exps_even = None
            for c in range(NCHUNK):
                for p in range(NPAIR):
                    exps = emit_s_exp(c, p)
                    o_sbs = emit_pv(c, p, exps)
                    if p % 2 == 0:
                        o_sbs_even = o_sbs
                    else:
                        q = p // 2
                        o_sb4 = (o_sbs_even[0], o_sbs_even[1],
                                 o_sbs[0], o_sbs[1])
                        rd = emit_recip(c, q, o_sb4)
                        pend.append((c, q, rd, o_sb4))
                    if c + 1 < NCHUNK:
                        emit_qproj(c + 1, p)
                    if len(pend) > 1:
                        emit_norm(pend.popleft())
                    if p == 2 and c > 0:
                        while pend and pend[0][0] < c:
                            emit_norm(pend.popleft())
                        emit_outproj(c - 1)
            while pend:
                emit_norm(pend.popleft())
            emit_outproj(NCHUNK - 1)

    nc.compile()
    return nc


def _get_nc():
    global _NC_CACHE
    if _NC_CACHE is None:
        _NC_CACHE = _build()
    return _NC_CACHE


def _make_in_maps(x, w_qkv, w_out):
    ones = np.ones((P, HD), dtype=np.float16)
    sel = np.zeros((P, 2, P), dtype=np.float16)
    for i in range(2):
        sel[64 * i, i, 0:64] = 1.0
        sel[64 * i + 32, i, 64:128] = 1.0
    per_g = []
    for g in range(2):
        qk_g = np.concatenate([w_qkv[g * 512:(g + 1) * 512],
                               w_qkv[DI + g * 512:DI + (g + 1) * 512]], axis=0)
        wqkT = np.ascontiguousarray(qk_g.T)               # [1024 d, 1024 f]
        wqkA = np.ascontiguousarray(
            wqkT.reshape(KT, P, 8, P).transpose(2, 1, 0, 3).astype(np.float16))
        v_g = w_qkv[2 * DI + g * 512:2 * DI + (g + 1) * 512]
        wvT = np.ascontiguousarray(v_g.T)                 # [1024 d, 512 f]
        wvA = np.ascontiguousarray(
            wvT.reshape(KT, P, 512).transpose(1, 0, 2).astype(np.float16))
        woTg = np.ascontiguousarray(
            w_out[:, g * 512:(g + 1) * 512].T.astype(np.float16))
        per_g.append((wqkA, wvA, woTg))

    in_maps = []
    for c in range(8):
        b, g = c // 2, c % 2
        wqkA, wvA, woTg = per_g[g]
        in_maps.append({
            "xT": np.ascontiguousarray(x[b].T.astype(np.float16)),
            "wqkA": wqkA,
            "wvA": wvA,
            "woT": woTg,
            "ones": ones,
            "sel": sel,
        })
    return in_maps


def kernel(x, w_qkv, w_out, b_out):
    x = np.asarray(x, dtype=np.float32)
    w_qkv = np.asarray(w_qkv, dtype=np.float32)
    w_out = np.asarray(w_out, dtype=np.float32)
    b_out = np.asarray(b_out, dtype=np.float32)
    B = x.shape[0]

    in_maps = _make_in_maps(x, w_qkv, w_out)
    nc = _get_nc()
    res = run_bass_kernel_spmd(nc, in_maps, core_ids=list(range(8)))
    parts = [r["outT"] for r in res.results]
    out = np.empty((B, N, DI), dtype=np.float32)
    for b in range(B):
        out[b] = (parts[2 * b].astype(np.float32)
                  + parts[2 * b + 1].astype(np.float32)).T + b_out
    return out


# revision 15
# speedup vs baseline: 1.2328x; 1.0016x over previous
"""Multi-head attention Trainium2 kernel (B=4, N=2048, D=1024, H=16).

Sharding: 8 cores = 4 batches x 2 head-groups (8 heads each), zero
collectives. Each core (all f16 compute, f32 PSUM accumulation):
  - projections: k/v + first q chunk up front; later q chunks projected
    mid-attention as PE gap filler
  - q,k kept transposed [feat, seq]; v row-layout [key, head, hd]
  - S matmuls packed two heads per pass via disjoint PE row groups into one
    [128,1024] PSUM tile
  - exp split between ACT (exact, table exp) and DVE (Schraudolph bit-trick:
    one tensor_scalar f32->int16 writing f16 bits; ~3% per-element error on
    a tunable fraction of key-tiles)
  - PV packed two heads per pass via disjoint PE col groups (M=64 each) at
    full array rate
  - softmax denominators via 4-way col-group-packed M=1 ones-matmuls
    (one PE pass covers 4 heads), reciprocal on the raw [1,512] rows,
    broadcast across partitions on the idle GPSIMD engine, normalize on DVE
  - out-projection partials staged f16; host sums the two head-group
    partials per batch and adds bias
"""
from collections import deque
from contextlib import ExitStack

import numpy as np

import concourse.mybir as mybir
import concourse.tile as tile
from concourse import bacc
from concourse.bass_utils import run_bass_kernel_spmd

F32 = mybir.dt.float32
F16 = mybir.dt.float16
I16 = mybir.dt.int16

P = 128
N = 2048         # sequence length
DI = 1024        # model dim
NH = 8           # heads per core
HD = 64          # head dim
NPAIR = 4        # head pairs per core
KT = 8           # contraction tiles for projections
CH = 512         # query chunk width
NCHUNK = 4       # chunks per sequence
MT = 16          # key tiles (m) per sequence
ET = 8           # output-feature blocks
SCALE = HD ** -0.5

LOG2E = 1.4426950408889634
A_SCH = SCALE * 1024.0 * LOG2E
B_SCH = 15.0 * 1024.0 - 55.0 + 0.5

# key-tile indices whose exp runs on DVE (Schraudolph approx); rest on ACT
DVE_EXP_MS = frozenset(m for m in range(MT) if m % 8 in (1, 4, 6))

_NC_CACHE = None


def _build():
    nc = bacc.Bacc("TRN2", target_bir_lowering=False, debug=False)

    xT = nc.dram_tensor("xT", [DI, N], F16, kind="ExternalInput").ap()
    wqkA = nc.dram_tensor("wqkA", [8, P, KT, P], F16, kind="ExternalInput").ap()
    wvA = nc.dram_tensor("wvA", [P, KT, 512], F16, kind="ExternalInput").ap()
    woT = nc.dram_tensor("woT", [512, DI], F16, kind="ExternalInput").ap()
    onesd = nc.dram_tensor("ones", [P, HD], F16, kind="ExternalInput").ap()
    seld = nc.dram_tensor("sel", [P, 2, P], F16, kind="ExternalInput").ap()
    outT = nc.dram_tensor("outT", [DI, N], F16, kind="ExternalOutput").ap()

    xT_r = xT.rearrange("(k p) n -> k p n", p=P)        # [8, 128, 2048]
    woT_r = woT.rearrange("(k p) e -> k p e", p=P)      # [4, 128, 1024]
    outT_r = outT.rearrange("(e p) n -> e p n", p=P)    # [8, 128, 2048]

    with tile.TileContext(nc) as tc, ExitStack() as persist:
        qk_pool = persist.enter_context(tc.tile_pool(name="qkp", bufs=8))
        v_pool = persist.enter_context(tc.tile_pool(name="vp", bufs=1))
        misc = persist.enter_context(tc.tile_pool(name="misc", bufs=1))
        xt_pool = persist.enter_context(tc.tile_pool(name="xt", bufs=8))
        wk_pool = persist.enter_context(tc.tile_pool(name="wk", bufs=4))
        wq_pool = persist.enter_context(tc.tile_pool(name="wq", bufs=4))
        wo_pool = persist.enter_context(tc.tile_pool(name="wo", bufs=4))

        ones_r = misc.tile([P, HD], F16)
        nc.sync.dma_start(ones_r[:], onesd[:])
        # selector weights: sel[:, i, :].T @ rd broadcasts rd row 64i across
        # partitions 0:64 and row 64i+32 across partitions 64:128
        sel_r = misc.tile([P, 2, P], F16, name="sel")
        nc.sync.dma_start(sel_r[:], seld[:])
        # warm the ACT exp table set early (one-time ~2.7us load)
        warm = misc.tile([1, 4], F16, name="warm")
        nc.scalar.activation(warm[:], ones_r[0:1, 0:4],
                             mybir.ActivationFunctionType.Exp)

        # DMA order: wk0 then x tiles (first matmul group needs wk0 + all
        # xt), with the remaining small weight tiles interleaved
        wk = [wk_pool.tile([P, KT, P], F16, name=f"wk{f}", tag="wk")
              for f in range(4)]
        xt = [xt_pool.tile([P, N], F16, name=f"xt{k}", tag="xt")
              for k in range(KT)]
        nc.sync.dma_start(wk[0][:], wqkA[4])
        for k in range(KT):
            nc.sync.dma_start(xt[k][:], xT_r[k])
            if 1 <= k <= 3:
                nc.sync.dma_start(wk[k][:], wqkA[4 + k])

        # qkT tiles: 0..3 = q head-pairs, 4..7 = k head-pairs.
        # Tile j holds heads 2j (parts 0:64) and 2j+1 (parts 64:128).
        qkT = [qk_pool.tile([P, N], F16, name=f"qkT{t}", tag="qkT")
               for t in range(8)]
        v_t = v_pool.tile([P, MT, NH, HD + 1], F16)
        nc.vector.tensor_copy(v_t[:, :, :, HD:HD + 1],
                              ones_r[:, 0:1].to_broadcast((P, MT, NH, 1)))

        # ---------------- Phase 1: k/v projections + first q chunk --------
        with ExitStack() as ph1:
            wv_pool = ph1.enter_context(tc.tile_pool(name="wv", bufs=1))
            pp = ph1.enter_context(tc.tile_pool(name="pp", bufs=4, space="PSUM"))

            wv = wv_pool.tile([P, KT, 512], F16)
            nc.sync.dma_start(wv[:], wvA[:])
            wq = []
            for f in range(4):
                t = wq_pool.tile([P, KT, P], F16, name=f"wq{f}", tag="wq")
                nc.sync.dma_start(t[:], wqkA[f])
                wq.append(t)
            wo = [wo_pool.tile([P, DI], F16, name=f"wo{kk}", tag="wo")
                  for kk in range(NPAIR)]
            for kk in range(NPAIR):
                nc.sync.dma_start(wo[kk][:], woT_r[kk])

            # k projection: feature blocks 4..7, all chunks
            for f in range(4):
                for c in range(NCHUNK):
                    ps = pp.tile([P, CH], F32, tag="pp")
                    for k in range(KT):
                        nc.tensor.matmul(ps[:], wk[f][:, k, :],
                                         xt[k][:, c * CH:(c + 1) * CH],
                                         start=(k == 0), stop=(k == KT - 1))
                    nc.vector.tensor_copy(qkT[4 + f][:, c * CH:(c + 1) * CH],
                                          ps[:])

            # v projection: row block r
            for r in range(MT):
                ps = pp.tile([P, CH], F32, tag="pp")
                for k in range(KT):
                    nc.tensor.matmul(ps[:], xt[k][:, r * P:(r + 1) * P],
                                     wv[:, k, :],
                                     start=(k == 0), stop=(k == KT - 1))
                nc.vector.tensor_copy(v_t[:, r, :, 0:HD],
                                      ps.rearrange("p (h d) -> p h d", d=HD))

            # q projection for chunk 0 only
            for f in range(4):
                ps = pp.tile([P, CH], F32, tag="pp")
                for k in range(KT):
                    nc.tensor.matmul(ps[:], wq[f][:, k, :], xt[k][:, 0:CH],
                                     start=(k == 0), stop=(k == KT - 1))
                nc.vector.tensor_copy(qkT[f][:, 0:CH], ps[:])

        # ---------------- Phase 2: attention + out-projection -------------
        with ExitStack() as ph2:
            exp_pool = ph2.enter_context(tc.tile_pool(name="expp", bufs=32))
            osb_pool = ph2.enter_context(tc.tile_pool(name="osb", bufs=4))
            ot_pool = ph2.enter_context(tc.tile_pool(name="ot", bufs=8))
            rd_pool = ph2.enter_context(tc.tile_pool(name="rd", bufs=2))
            st_pool = ph2.enter_context(tc.tile_pool(name="stg", bufs=2))
            sps_pool = ph2.enter_context(
                tc.tile_pool(name="sps", bufs=2, space="PSUM"))
            oaug_pool = ph2.enter_context(
                tc.tile_pool(name="oaug", bufs=1, space="PSUM"))
            aux_pool = ph2.enter_context(
                tc.tile_pool(name="aux", bufs=2, space="PSUM"))

            ot_map = {}

            def emit_s_exp(c, p):
                csl = slice(c * CH, (c + 1) * CH)
                qA = qkT[p][0:HD, csl]
                qB = qkT[p][HD:P, csl]
                kTl = qkT[4 + p]
                exps = []
                for m in range(MT):
                    msl = slice(m * P, (m + 1) * P)
                    s_ps = sps_pool.tile([P, 2 * CH], F32, tag="sps",
                                         name=f"sps_{c}_{p}_{m}")
                    nc.tensor.matmul(s_ps[:, 0:CH], kTl[0:HD, msl], qA,
                                     start=True, stop=True)
                    nc.tensor.matmul(s_ps[:, CH:2 * CH], kTl[HD:P, msl], qB,
                                     start=True, stop=True)
                    e = exp_pool.tile([P, 2 * CH], F16, tag="expp",
                                      name=f"expP_{c}_{p}_{m}")
                    if m in DVE_EXP_MS:
                        nc.vector.tensor_scalar(
                            e[:].bitcast(I16), s_ps[:], A_SCH, B_SCH,
                            mybir.AluOpType.mult, mybir.AluOpType.add)
                    else:
                        nc.scalar.activation(
                            e[:], s_ps[:], mybir.ActivationFunctionType.Exp,
                            scale=SCALE)
                    exps.append(e)
                return exps

            def emit_pv(c, p, exps):
                # M=65 aug: the ones column of v makes output row 64 the
                # softmax denominator for free; two heads sequential
                o_sbs = []
                for h in range(2):
                    oaug = oaug_pool.tile([HD + 1, CH], F32, tag="oaug",
                                          name=f"oaug_{c}_{p}_{h}")
                    for m in range(MT):
                        nc.tensor.matmul(oaug[:], v_t[:, m, 2 * p + h, :],
                                         exps[m][:, h * CH:(h + 1) * CH],
                                         start=(m == 0), stop=(m == MT - 1))
                    o_sb = osb_pool.tile([HD + 1, CH], F32, tag="osb",
                                         name=f"osb_{c}_{p}_{h}")
                    nc.scalar.copy(o_sb[:], oaug[:])
                    o_sbs.append(o_sb)
                return o_sbs

            def emit_recip(c, q, o_sb4):
                # gather the 4 heads' raw denominators (row 64 of each evac)
                # into one tile, then a single reciprocal: the instruction's
                # ~3.4us fixed cost dominates, so one [128,512] beats four
                rd = rd_pool.tile([P, CH], F16, tag="rd", name=f"rdr_{c}_{q}")
                nc.vector.memset(rd[:], 1.0)
                for j, osb in enumerate(o_sb4):
                    nc.vector.tensor_copy(rd[32 * j:32 * j + 1, :],
                                          osb[HD:HD + 1, :])
                rd2 = rd_pool.tile([P, CH], F16, tag="rd2", name=f"rd_{c}_{q}")
                with nc.allow_low_precision(reason="softmax denom recip"):
                    nc.vector.reciprocal(rd2[:], rd[:])
                return rd2

            def emit_norm(unit):
                c, q, rd, o_sb4 = unit
                for i, pp_ in enumerate((2 * q, 2 * q + 1)):
                    rbc = aux_pool.tile([P, CH], F32, tag="aux",
                                        name=f"rbc_{c}_{q}_{i}")
                    nc.tensor.matmul(rbc[:], sel_r[:, i, :], rd[:],
                                     start=True, stop=True)
                    ot_p = ot_pool.tile([P, CH], F16, tag="ot",
                                        name=f"ot_{c}_{pp_}")
                    nc.vector.tensor_tensor(ot_p[0:HD, :],
                                            o_sb4[2 * i][0:HD, :],
                                            rbc[0:HD, :],
                                            mybir.AluOpType.mult)
                    nc.vector.tensor_tensor(ot_p[HD:P, :],
                                            o_sb4[2 * i + 1][0:HD, :],
                                            rbc[HD:P, :],
                                            mybir.AluOpType.mult)
                    ot_map[(c, pp_)] = ot_p

            def emit_qproj(c, f):
                csl = slice(c * CH, (c + 1) * CH)
                ps = aux_pool.tile([P, CH], F32, tag="aux",
                                   name=f"qp_{c}_{f}")
                for k in range(KT):
                    nc.tensor.matmul(ps[:], wq[f][:, k, :], xt[k][:, csl],
                                     start=(k == 0), stop=(k == KT - 1))
                nc.vector.tensor_copy(qkT[f][:, csl], ps[:])

            def emit_outproj(c):
                csl = slice(c * CH, (c + 1) * CH)
                for e in range(ET):
                    pso = aux_pool.tile([P, CH], F32, tag="aux",
                                        name=f"pso_{c}_{e}")
                    for p in range(NPAIR):
                        nc.tensor.matmul(pso[:],
                                         wo[p][:, e * P:(e + 1) * P],
                                         ot_map[(c, p)][:],
                                         start=(p == 0), stop=(p == NPAIR - 1))
                    st = st_pool.tile([P, CH], F16, tag="stg",
                                      name=f"st_{c}_{e}")
                    nc.scalar.copy(st[:], pso[:])
                    nc.sync.dma_start(outT_r[e][:, csl], st[:])

            # software pipeline: quad q's norm is emitted one unit after its
            # den completes; chunk c's out-projection after chunk c+1's first
            # unit, so the PE queue never blocks on the DVE/GPSIMD chain
            pend = deque()
            exps_even = None
            for c in range(NCHUNK):
                for p in range(NPAIR):
                    # lagged norm first: its DVE mult and PE bcast enter the
                    # engine queues ahead of this unit's exps/matmuls, so the
                    # aux-pool ring never stalls the PE on a queued reader
                    if len(pend) > 1:
                        emit_norm(pend.popleft())
                    if p == 2 and c > 0:
                        while pend and pend[0][0] < c:
                            emit_norm(pend.popleft())
                        emit_outproj(c - 1)
                    exps = emit_s_exp(c, p)
                    o_sbs = emit_pv(c, p, exps)
                    if p % 2 == 0:
                        o_sbs_even = o_sbs
                    else:
                        q = p // 2
                        o_sb4 = (o_sbs_even[0], o_sbs_even[1],
                                 o_sbs[0], o_sbs[1])
                        rd = emit_recip(c, q, o_sb4)
                        pend.append((c, q, rd, o_sb4))
                    if c + 1 < NCHUNK:
                        emit_qproj(c + 1, p)
            while pend:
                emit_norm(pend.popleft())
            emit_outproj(NCHUNK - 1)

    nc.compile()
    return nc


def _get_nc():
    global _NC_CACHE
    if _NC_CACHE is None:
        _NC_CACHE = _build()
    return _NC_CACHE


def _make_in_maps(x, w_qkv, w_out):
    ones = np.ones((P, HD), dtype=np.float16)
    sel = np.zeros((P, 2, P), dtype=np.float16)
    for i in range(2):
        sel[64 * i, i, 0:64] = 1.0
        sel[64 * i + 32, i, 64:128] = 1.0
    per_g = []
    for g in range(2):
        qk_g = np.concatenate([w_qkv[g * 512:(g + 1) * 512],
                               w_qkv[DI + g * 512:DI + (g + 1) * 512]], axis=0)
        wqkT = np.ascontiguousarray(qk_g.T)               # [1024 d, 1024 f]
        wqkA = np.ascontiguousarray(
            wqkT.reshape(KT, P, 8, P).transpose(2, 1, 0, 3).astype(np.float16))
        v_g = w_qkv[2 * DI + g * 512:2 * DI + (g + 1) * 512]
        wvT = np.ascontiguousarray(v_g.T)                 # [1024 d, 512 f]
        wvA = np.ascontiguousarray(
            wvT.reshape(KT, P, 512).transpose(1, 0, 2).astype(np.float16))
        woTg = np.ascontiguousarray(
            w_out[:, g * 512:(g + 1) * 512].T.astype(np.float16))
        per_g.append((wqkA, wvA, woTg))

    in_maps = []
    for c in range(8):
        b, g = c // 2, c % 2
        wqkA, wvA, woTg = per_g[g]
        in_maps.append({
            "xT": np.ascontiguousarray(x[b].T.astype(np.float16)),
            "wqkA": wqkA,
            "wvA": wvA,
            "woT": woTg,
            "ones": ones,
            "sel": sel,
        })
    return in_maps


def kernel(x, w_qkv, w_out, b_out):
    x = np.asarray(x, dtype=np.float32)
    w_qkv = np.asarray(w_qkv, dtype=np.float32)
    w_out = np.asarray(w_out, dtype=np.float32)
    b_out = np.asarray(b_out, dtype=np.float32)
    B = x.shape[0]

    in_maps = _make_in_maps(x, w_qkv, w_out)
    nc = _get_nc()
    res = run_bass_kernel_spmd(nc, in_maps, core_ids=list(range(8)))
    parts = [r["outT"] for r in res.results]
    out = np.empty((B, N, DI), dtype=np.float32)
    for b in range(B):
        out[b] = (parts[2 * b].astype(np.float32)
                  + parts[2 * b + 1].astype(np.float32)).T + b_out
    return out
